# revision 10
# baseline (speedup 1.0000x reference)
"""Multi-head attention (B=2, T=2048, D=1024, H=16, Dh=64) on 8 TRN2 NeuronCores.

Sharding: core c = 4*b + g  ->  batch b in {0,1}, head-group g in {0..3}
(4 heads per core: data parallel on batch, tensor parallel on heads).
Each core computes, for its batch element and its 4 heads:

  Q.T/K.T = Wq/k_shard.T @ x.T + b      [256, 2048]  (head-dim on partitions)
  V       = x @ Wv_shard + b            [2048, 256]  (tokens on partitions)
  per head h:
    S.T   = K_h Q_h.T                   [2048 j, 2048 i]   (scores transposed)
    P.T   = exp(S.T / 8)                (no max-subtraction: |S|/8 <~ 6, safe)
    acc   = [V_h | 1].T @ P.T           [65, 2048]   row 64 = softmax denom
    attnT = acc[:64] * (1/acc[64])      [64, 2048]
  partial = attnT.T @ Wout_shard        [2048, 1024]

The 4-way partial sum over head groups plus b_out is done on the host
(the "all-reduce after out_proj"), as is the batch unshard.

All matmuls use float32r (full fp32 storage, fast PE path).
"""

import os
import numpy as np

B, T, D = 2, 2048, 1024
H, DH = 16, 64
NCORES, GROUPS = 8, 4
HPC = H // GROUPS        # 4 heads per core
F = HPC * DH             # 256 features per core
FT = F // 128            # 2 feature tiles
KTN = D // 128           # 8 contraction tiles
TT = T // 128            # 16 token tiles
NCH = 512                # matmul free-dim chunk
IH = 1024                # attention i-half (psum budget)
VW = DH + 1              # 65: V plus ones column
VF = HPC * VW            # 260: interleaved [V_h | 1] x 4 heads

_prog = None
LAST_RESULT = None


def _build():
    from contextlib import ExitStack

    import concourse.mybir as mybir
    import concourse.tile as tile
    from concourse import bacc
    from concourse.bass import ts

    f32 = mybir.dt.float32
    f32r = mybir.dt.float32r
    Exp = mybir.ActivationFunctionType.Exp

    nc = bacc.Bacc()
    xT = nc.dram_tensor("xT", [D, T], f32r, kind="ExternalInput")
    wq = nc.dram_tensor("wq", [D, F], f32r, kind="ExternalInput")
    wk = nc.dram_tensor("wk", [D, F], f32r, kind="ExternalInput")
    # wv/bv come pre-interleaved from the host: column h*65+64 is a zero
    # weight column whose bias is 1.0, producing the [V_h | 1] layout that
    # supplies the softmax-denominator row of the PV matmul for free.
    wv = nc.dram_tensor("wv", [D, VF], f32r, kind="ExternalInput")
    bq = nc.dram_tensor("bq", [F, 1], f32, kind="ExternalInput")
    bk = nc.dram_tensor("bk", [F, 1], f32, kind="ExternalInput")
    bv = nc.dram_tensor("bv", [1, VF], f32r, kind="ExternalInput")
    wo = nc.dram_tensor("wo", [F, D], f32r, kind="ExternalInput")
    out = nc.dram_tensor("out", [T, D], f32, kind="ExternalOutput")

    with ExitStack() as ctx:
        tc = ctx.enter_context(tile.TileContext(nc))
        pers = ctx.enter_context(tc.tile_pool(name="pers", bufs=1))
        ptp = ctx.enter_context(tc.tile_pool(name="ptp", bufs=2))
        osb = ctx.enter_context(tc.tile_pool(name="osb", bufs=2))
        msc = ctx.enter_context(tc.tile_pool(name="msc", bufs=2))
        psq = ctx.enter_context(tc.tile_pool(name="psq", bufs=2, space="PSUM"))
        pss = ctx.enter_context(tc.tile_pool(name="pss", bufs=2, space="PSUM"))
        pso = ctx.enter_context(tc.tile_pool(name="pso", bufs=1, space="PSUM"))

        xt = pers.tile([128, KTN, T], f32r, tag="xt")
        wqs = pers.tile([128, KTN, F], f32r, tag="wqs")
        wks = pers.tile([128, KTN, F], f32r, tag="wks")
        wvs = pers.tile([128, KTN, VF], f32r, tag="wvs")
        bqc = pers.tile([128, FT, 1], f32, tag="bqc")
        bkc = pers.tile([128, FT, 1], f32, tag="bkc")
        bvr = pers.tile([1, VF], f32r, tag="bvr")
        ones_f = pers.tile([1, 128], f32, tag="ones_f")
        ones = pers.tile([1, 128], f32r, tag="ones")
        wos = pers.tile([128, FT, D], f32r, tag="wos")
        qt = pers.tile([128, FT, T], f32r, tag="qt")
        kt = pers.tile([128, FT, T], f32r, tag="kt")
        vs = pers.tile([128, TT, VF], f32r, tag="vs")
        at = pers.tile([128, FT, T], f32r, tag="at")

        # ISA memset can't target f32r; memset f32 then copy-convert
        nc.vector.memset(ones_f[:], 1.0)
        nc.vector.tensor_copy(ones[:], ones_f[:])

        nc.sync.dma_start(bvr[:], bv[:])
        for ft in range(FT):
            nc.sync.dma_start(bqc[:, ft, :], bq[ts(ft, 128), :])
            nc.sync.dma_start(bkc[:, ft, :], bk[ts(ft, 128), :])
        for k in range(KTN):
            nc.sync.dma_start(wqs[:, k, :], wq[ts(k, 128), :])
            nc.sync.dma_start(wks[:, k, :], wk[ts(k, 128), :])
            nc.sync.dma_start(wvs[:, k, :], wv[ts(k, 128), :])
            nc.sync.dma_start(xt[:, k, :], xT[ts(k, 128), :])
        for ft in range(FT):
            nc.sync.dma_start(wos[:, ft, :], wo[ts(ft, 128), :])

        # ---- Q.T / K.T projections: [256 f, 2048 t], f on partitions ----
        for wsb, bcol, dst in ((wqs, bqc, qt), (wks, bkc, kt)):
            for ft in range(FT):
                for c in range(T // NCH):
                    ps = psq.tile([128, NCH], f32, tag="psq")
                    for k in range(KTN):
                        nc.tensor.matmul(
                            ps[:],
                            wsb[:, k, ts(ft, 128)],
                            xt[:, k, ts(c, NCH)],
                            start=(k == 0),
                            stop=(k == KTN - 1),
                        )
                    nc.vector.tensor_scalar_add(
                        dst[:, ft, ts(c, NCH)], ps[:], bcol[:, ft, :]
                    )

        # ---- V projection: [2048 t, 260], tokens on partitions ----
        for t in range(TT):
            pv = psq.tile([128, VF], f32, tag="psq")
            for k in range(KTN):
                nc.tensor.matmul(
                    pv[:], xt[:, k, ts(t, 128)], wvs[:, k, :],
                    start=(k == 0), stop=False,
                )
            # bias via ones-row: V += ones.T @ bv (also writes the 1.0 cols)
            nc.tensor.matmul(pv[:], ones[:, 0:128], bvr[:], start=False, stop=True)
            nc.vector.tensor_copy(vs[:, t, :], pv[:])

        # ---- attention ----
        for h in range(HPC):
            ht, hp = h // 2, (h % 2) * 64
            for i in range(T // IH):
                acc = pso.tile([VW, IH], f32, tag="pso")

                def scores(j):
                    sc = pss.tile([128, IH], f32, tag="sc")
                    for c in range(IH // NCH):
                        nc.tensor.matmul(
                            sc[:, ts(c, NCH)],
                            kt[hp: hp + DH, ht, ts(j, 128)],
                            qt[hp: hp + DH, ht,
                               i * IH + c * NCH: i * IH + (c + 1) * NCH],
                            start=True, stop=True,
                        )
                    return sc

                sc_cur = scores(0)
                for j in range(TT):
                    pe = ptp.tile([128, IH], f32r, tag="pe")
                    nc.scalar.activation(pe[:], sc_cur[:], Exp, scale=0.125)
                    if j + 1 < TT:
                        sc_cur = scores(j + 1)  # keep PE busy during exp
                    for c in range(IH // NCH):
                        nc.tensor.matmul(
                            acc[:, ts(c, NCH)],
                            vs[:, j, h * VW: (h + 1) * VW],
                            pe[:, ts(c, NCH)],
                            start=(j == 0), stop=(j == TT - 1),
                        )

                # normalize: attnT = acc[:64] * (1 / acc[64]) , bcast via PE
                rc = msc.tile([1, IH], f32r, tag="rc", bufs=2)
                with nc.allow_low_precision(reason="f32r is 4-byte fp32 storage"):
                    nc.vector.reciprocal(rc[:], acc[DH: DH + 1, :])
                for c in range(IH // NCH):
                    pb = psq.tile([64, NCH], f32, tag="psq")
                    nc.tensor.matmul(
                        pb[:], ones[:, 0:64], rc[:, ts(c, NCH)],
                        start=True, stop=True,
                    )
                    bsb = msc.tile([64, NCH], f32, tag="bsb")
                    nc.vector.tensor_copy(bsb[:], pb[:])
                    dst_sl = slice(i * IH + c * NCH, i * IH + (c + 1) * NCH)
                    if hp == 0:
                        nc.vector.tensor_mul(
                            at[0:DH, ht, dst_sl], acc[0:DH, ts(c, NCH)], bsb[:]
                        )
                    else:
                        # DVE lanes can't shift partitions; bounce via DMA
                        tmp = msc.tile([DH, NCH], f32r, tag="tmp", bufs=2)
                        nc.vector.tensor_mul(
                            tmp[:], acc[0:DH, ts(c, NCH)], bsb[:]
                        )
                        nc.sync.dma_start(at[64:128, ht, dst_sl], tmp[:])

        # ---- out projection: partial = attnT.T @ Wout_shard ----
        for t in range(TT):
            ob = osb.tile([128, D], f32, tag="ob")
            for c in range(D // NCH):
                pp = psq.tile([128, NCH], f32, tag="psq")
                for ft in range(FT):
                    nc.tensor.matmul(
                        pp[:],
                        at[:, ft, ts(t, 128)],
                        wos[:, ft, ts(c, NCH)],
                        start=(ft == 0), stop=(ft == FT - 1),
                    )
                nc.vector.tensor_copy(ob[:, ts(c, NCH)], pp[:])
            nc.sync.dma_start(out[ts(t, 128), :], ob[:])

    nc.finalize()  # Bacc.compile(): wait legalization, reg alloc, act tables
    return nc


def _get_program():
    global _prog
    if _prog is None:
        _prog = _build()
    return _prog


def kernel(x, W_qkv, b_qkv, W_out, b_out):
    global LAST_RESULT
    from concourse.bass_utils import run_bass_kernel_spmd

    x = np.asarray(x, np.float32)
    W_qkv = np.asarray(W_qkv, np.float32)
    b_qkv = np.asarray(b_qkv, np.float32)
    W_out = np.asarray(W_out, np.float32)
    b_out = np.asarray(b_out, np.float32)

    nc = _get_program()

    in_maps = []
    for c in range(NCORES):
        b, g = divmod(c, GROUPS)
        sl = slice(g * F, (g + 1) * F)
        # interleave Wv/bv with [zero-weight, bias=1] columns at h*65+64
        wv_g = W_qkv[:, 2 * D:3 * D][:, sl]
        bv_g = b_qkv[2 * D:3 * D][sl]
        wv_i = np.zeros((D, VF), np.float32)
        bv_i = np.zeros((1, VF), np.float32)
        for h in range(HPC):
            wv_i[:, h * VW: h * VW + DH] = wv_g[:, h * DH:(h + 1) * DH]
            bv_i[0, h * VW: h * VW + DH] = bv_g[h * DH:(h + 1) * DH]
            bv_i[0, h * VW + DH] = 1.0
        in_maps.append({
            "xT": np.ascontiguousarray(x[b].T),
            "wq": np.ascontiguousarray(W_qkv[:, 0 * D:1 * D][:, sl]),
            "wk": np.ascontiguousarray(W_qkv[:, 1 * D:2 * D][:, sl]),
            "wv": wv_i,
            "bq": np.ascontiguousarray(b_qkv[0 * D:1 * D][sl][:, None]),
            "bk": np.ascontiguousarray(b_qkv[1 * D:2 * D][sl][:, None]),
            "bv": bv_i,
            "wo": np.ascontiguousarray(W_out[sl, :]),
        })

    kw = {}
    if os.environ.get("KERNEL_TRACE") == "1":
        kw["trace"] = True
    res = run_bass_kernel_spmd(nc, in_maps, core_ids=list(range(NCORES)), **kw)
    LAST_RESULT = res

    out = np.empty((B, T, D), np.float32)
    for b in range(B):
        acc = res.results[GROUPS * b]["out"].astype(np.float32)
        for g in range(1, GROUPS):
            acc = acc + res.results[GROUPS * b + g]["out"]
        out[b] = acc + b_out
    return out


# revision 13
# speedup vs baseline: 1.2504x; 1.2504x over previous
"""Multi-head attention (B=2, T=2048, D=1024, H=16, Dh=64) on 8 TRN2 NeuronCores.

Sharding: core c = 4*b + g  ->  batch b in {0,1}, head-group g in {0..3}
(4 heads per core: data parallel on batch, tensor parallel on heads).
Each core computes, for its batch element and its 4 heads:

  Q.T/K.T = Wq/k_shard.T @ x.T + b      [256, 2048]  (head-dim on partitions)
  V       = x @ Wv_shard + b            [2048, 256]  (tokens on partitions)
  per head h:
    S.T   = K_h Q_h.T                   [2048 j, 2048 i]   (scores transposed)
    P.T   = exp(S.T / 8)                (no max-subtraction: |S|/8 <~ 6, safe)
    acc   = [V_h | 1].T @ P.T           [65, 2048]   row 64 = softmax denom
    attnT = acc[:64] * (1/acc[64])      [64, 2048]
  partial = attnT.T @ Wout_shard        [2048, 1024]

The 4-way partial sum over head groups plus b_out is done on the host
(the "all-reduce after out_proj"), as is the batch unshard.

All matmuls use float32r (full fp32 storage, fast PE path).
"""

import os
import numpy as np

B, T, D = 2, 2048, 1024
H, DH = 16, 64
NCORES, GROUPS = 8, 4
HPC = H // GROUPS        # 4 heads per core
F = HPC * DH             # 256 features per core
FT = F // 128            # 2 feature tiles
KTN = D // 128           # 8 contraction tiles
TT = T // 128            # 16 token tiles
NCH = 512                # matmul free-dim chunk
IH = 1024                # attention i-half (psum budget)
VW = DH + 1              # 65: V plus ones column
VF = HPC * VW            # 260: interleaved [V_h | 1] x 4 heads

_prog = None
LAST_RESULT = None


def _build():
    from contextlib import ExitStack

    import concourse.mybir as mybir
    import concourse.tile as tile
    from concourse import bacc
    from concourse.bass import ts

    f32 = mybir.dt.float32
    f32r = mybir.dt.float32r
    Exp = mybir.ActivationFunctionType.Exp

    nc = bacc.Bacc()
    xT = nc.dram_tensor("xT", [D, T], f32r, kind="ExternalInput")
    wq = nc.dram_tensor("wq", [D, F], f32r, kind="ExternalInput")
    wk = nc.dram_tensor("wk", [D, F], f32r, kind="ExternalInput")
    # wv/bv come pre-interleaved from the host: column h*65+64 is a zero
    # weight column whose bias is 1.0, producing the [V_h | 1] layout that
    # supplies the softmax-denominator row of the PV matmul for free.
    wv = nc.dram_tensor("wv", [D, VF], f32r, kind="ExternalInput")
    bq = nc.dram_tensor("bq", [F, 1], f32, kind="ExternalInput")
    bk = nc.dram_tensor("bk", [F, 1], f32, kind="ExternalInput")
    bv = nc.dram_tensor("bv", [1, VF], f32r, kind="ExternalInput")
    wo = nc.dram_tensor("wo", [F, D], f32r, kind="ExternalInput")
    out = nc.dram_tensor("out", [T, D], f32, kind="ExternalOutput")

    with ExitStack() as ctx:
        tc = ctx.enter_context(tile.TileContext(nc))
        pers = ctx.enter_context(tc.tile_pool(name="pers", bufs=1))
        ptp = ctx.enter_context(tc.tile_pool(name="ptp", bufs=2))
        osb = ctx.enter_context(tc.tile_pool(name="osb", bufs=2))
        msc = ctx.enter_context(tc.tile_pool(name="msc", bufs=2))
        psq = ctx.enter_context(tc.tile_pool(name="psq", bufs=2, space="PSUM"))
        pss = ctx.enter_context(tc.tile_pool(name="pss", bufs=2, space="PSUM"))
        pso = ctx.enter_context(tc.tile_pool(name="pso", bufs=1, space="PSUM"))

        xt = pers.tile([128, KTN, T], f32r, tag="xt")
        wqs = pers.tile([128, KTN, F], f32r, tag="wqs")
        wks = pers.tile([128, KTN, F], f32r, tag="wks")
        wvs = pers.tile([128, KTN, VF], f32r, tag="wvs")
        bqc = pers.tile([128, FT, 1], f32, tag="bqc")
        bkc = pers.tile([128, FT, 1], f32, tag="bkc")
        bvr = pers.tile([1, VF], f32r, tag="bvr")
        ones_f = pers.tile([1, 128], f32, tag="ones_f")
        ones = pers.tile([1, 128], f32r, tag="ones")
        wos = pers.tile([128, FT, D], f32r, tag="wos")
        qt = pers.tile([128, FT, T], f32r, tag="qt")
        kt = pers.tile([128, FT, T], f32r, tag="kt")
        vs = pers.tile([128, TT, VF], f32r, tag="vs")
        at = pers.tile([128, FT, T], f32r, tag="at")

        # ISA memset can't target f32r; memset f32 then copy-convert
        nc.vector.memset(ones_f[:], 1.0)
        nc.vector.tensor_copy(ones[:], ones_f[:])

        nc.sync.dma_start(bvr[:], bv[:])
        for ft in range(FT):
            nc.sync.dma_start(bqc[:, ft, :], bq[ts(ft, 128), :])
            nc.sync.dma_start(bkc[:, ft, :], bk[ts(ft, 128), :])
        for k in range(KTN):
            nc.sync.dma_start(wqs[:, k, :], wq[ts(k, 128), :])
            nc.sync.dma_start(wks[:, k, :], wk[ts(k, 128), :])
            nc.sync.dma_start(wvs[:, k, :], wv[ts(k, 128), :])
            nc.sync.dma_start(xt[:, k, :], xT[ts(k, 128), :])
        for ft in range(FT):
            nc.sync.dma_start(wos[:, ft, :], wo[ts(ft, 128), :])

        # ---- Q.T / K.T projections: [256 f, 2048 t], f on partitions ----
        for wsb, bcol, dst in ((wqs, bqc, qt), (wks, bkc, kt)):
            for ft in range(FT):
                for c in range(T // NCH):
                    ps = psq.tile([128, NCH], f32, tag="psq")
                    for k in range(KTN):
                        nc.tensor.matmul(
                            ps[:],
                            wsb[:, k, ts(ft, 128)],
                            xt[:, k, ts(c, NCH)],
                            start=(k == 0),
                            stop=(k == KTN - 1),
                        )
                    nc.vector.tensor_scalar_add(
                        dst[:, ft, ts(c, NCH)], ps[:], bcol[:, ft, :]
                    )

        # ---- V projection: [2048 t, 260], tokens on partitions ----
        for t in range(TT):
            pv = psq.tile([128, VF], f32, tag="psq")
            for k in range(KTN):
                nc.tensor.matmul(
                    pv[:], xt[:, k, ts(t, 128)], wvs[:, k, :],
                    start=(k == 0), stop=False,
                )
            # bias via ones-row: V += ones.T @ bv (also writes the 1.0 cols)
            nc.tensor.matmul(pv[:], ones[:, 0:128], bvr[:], start=False, stop=True)
            nc.vector.tensor_copy(vs[:, t, :], pv[:])

        # ---- attention: head pairs (fills both PE row-group halves) ----
        for p in range(FT):                 # pair p = heads (2p, 2p+1)
            for ic in range(T // NCH):      # 4 i-chunks of 512
                acc0 = pso.tile([VW, NCH], f32, tag="acc0")
                acc1 = pso.tile([VW, NCH], f32, tag="acc1")
                accs = (acc0, acc1)

                def scores(j):
                    # the two K=64 matmuls use disjoint PE row groups
                    # (partitions 0-63 vs 64-127) -> run concurrently
                    sc = pss.tile([128, 2 * NCH], f32, tag="sc")
                    for hh in range(2):
                        nc.tensor.matmul(
                            sc[:, ts(hh, NCH)],
                            kt[hh * 64: hh * 64 + DH, p, ts(j, 128)],
                            qt[hh * 64: hh * 64 + DH, p, ts(ic, NCH)],
                            start=True, stop=True,
                        )
                    return sc

                sc_cur = scores(0)
                for j in range(TT):
                    pe = ptp.tile([128, 2 * NCH], f32r, tag="pe")
                    nc.scalar.activation(pe[:], sc_cur[:], Exp, scale=0.125)
                    if j + 1 < TT:
                        sc_cur = scores(j + 1)  # keep PE busy during exp
                    for hh in range(2):
                        nc.tensor.matmul(
                            accs[hh][:, :],
                            vs[:, j, (2 * p + hh) * VW: (2 * p + hh + 1) * VW],
                            pe[:, ts(hh, NCH)],
                            start=(j == 0), stop=(j == TT - 1),
                        )

                # normalize: attnT = acc[:64] * (1 / acc[64]) , bcast via PE
                for hh in range(2):
                    acc = accs[hh]
                    # custom-DVE ops drop the partition base offset; stage the
                    # denominator row to SBUF partition 0 first
                    dn = msc.tile([1, NCH], f32, tag="dn", bufs=2)
                    nc.vector.tensor_copy(dn[:], acc[DH: DH + 1, :])
                    rc = msc.tile([1, NCH], f32, tag="rc", bufs=2)
                    nc.vector.reciprocal_approx_fast(rc[:], dn[:])
                    rcr = msc.tile([1, NCH], f32r, tag="rcr", bufs=2)
                    nc.vector.tensor_copy(rcr[:], rc[:])  # round to f32r
                    pb = psq.tile([64, NCH], f32, tag="psq")
                    nc.tensor.matmul(
                        pb[:], ones[:, 0:64], rcr[:],
                        start=True, stop=True,
                    )
                    bsb = msc.tile([64, NCH], f32, tag="bsb")
                    nc.vector.tensor_copy(bsb[:], pb[:])
                    dst_sl = ts(ic, NCH)
                    if hh == 0:
                        nc.vector.tensor_mul(
                            at[0:DH, p, dst_sl], acc[0:DH, :], bsb[:]
                        )
                    else:
                        # DVE lanes can't shift partitions; bounce via DMA
                        tmp = msc.tile([DH, NCH], f32r, tag="tmp", bufs=2)
                        nc.vector.tensor_mul(tmp[:], acc[0:DH, :], bsb[:])
                        nc.sync.dma_start(at[64:128, p, dst_sl], tmp[:])

        # ---- out projection: partial = attnT.T @ Wout_shard ----
        for t in range(TT):
            ob = osb.tile([128, D], f32, tag="ob")
            for c in range(D // NCH):
                pp = psq.tile([128, NCH], f32, tag="psq")
                for ft in range(FT):
                    nc.tensor.matmul(
                        pp[:],
                        at[:, ft, ts(t, 128)],
                        wos[:, ft, ts(c, NCH)],
                        start=(ft == 0), stop=(ft == FT - 1),
                    )
                nc.vector.tensor_copy(ob[:, ts(c, NCH)], pp[:])
            nc.sync.dma_start(out[ts(t, 128), :], ob[:])

    nc.finalize()  # Bacc.compile(): wait legalization, reg alloc, act tables
    return nc


def _get_program():
    global _prog
    if _prog is None:
        _prog = _build()
    return _prog


def kernel(x, W_qkv, b_qkv, W_out, b_out):
    global LAST_RESULT
    from concourse.bass_utils import run_bass_kernel_spmd

    x = np.asarray(x, np.float32)
    W_qkv = np.asarray(W_qkv, np.float32)
    b_qkv = np.asarray(b_qkv, np.float32)
    W_out = np.asarray(W_out, np.float32)
    b_out = np.asarray(b_out, np.float32)

    nc = _get_program()

    in_maps = []
    for c in range(NCORES):
        b, g = divmod(c, GROUPS)
        sl = slice(g * F, (g + 1) * F)
        # interleave Wv/bv with [zero-weight, bias=1] columns at h*65+64
        wv_g = W_qkv[:, 2 * D:3 * D][:, sl]
        bv_g = b_qkv[2 * D:3 * D][sl]
        wv_i = np.zeros((D, VF), np.float32)
        bv_i = np.zeros((1, VF), np.float32)
        for h in range(HPC):
            wv_i[:, h * VW: h * VW + DH] = wv_g[:, h * DH:(h + 1) * DH]
            bv_i[0, h * VW: h * VW + DH] = bv_g[h * DH:(h + 1) * DH]
            bv_i[0, h * VW + DH] = 1.0
        in_maps.append({
            "xT": np.ascontiguousarray(x[b].T),
            "wq": np.ascontiguousarray(W_qkv[:, 0 * D:1 * D][:, sl]),
            "wk": np.ascontiguousarray(W_qkv[:, 1 * D:2 * D][:, sl]),
            "wv": wv_i,
            "bq": np.ascontiguousarray(b_qkv[0 * D:1 * D][sl][:, None]),
            "bk": np.ascontiguousarray(b_qkv[1 * D:2 * D][sl][:, None]),
            "bv": bv_i,
            "wo": np.ascontiguousarray(W_out[sl, :]),
        })

    kw = {}
    if os.environ.get("KERNEL_TRACE") == "1":
        kw["trace"] = True
    res = run_bass_kernel_spmd(nc, in_maps, core_ids=list(range(NCORES)), **kw)
    LAST_RESULT = res

    out = np.empty((B, T, D), np.float32)
    for b in range(B):
        acc = res.results[GROUPS * b]["out"].astype(np.float32)
        for g in range(1, GROUPS):
            acc = acc + res.results[GROUPS * b + g]["out"]
        out[b] = acc + b_out
    return out


# revision 18
# speedup vs baseline: 1.4581x; 1.1661x over previous
"""Multi-head attention (B=2, T=2048, D=1024, H=16, Dh=64) on 8 TRN2 NeuronCores.

Sharding: core c = 4*b + g  ->  batch b in {0,1}, head-group g in {0..3}
(4 heads per core: data parallel on batch, tensor parallel on heads).
Each core computes, for its batch element and its 4 heads:

  Q.T/K.T = Wq/k_shard.T @ x.T + b      [256, 2048]  (head-dim on partitions)
  V       = x @ Wv_shard + b            [2048, 256]  (tokens on partitions)
  per head h:
    S.T   = K_h Q_h.T                   [2048 j, 2048 i]   (scores transposed)
    P.T   = exp(S.T / 8)                (no max-subtraction: |S|/8 <~ 6, safe)
    acc   = [V_h | 1].T @ P.T           [65, 2048]   row 64 = softmax denom
    attnT = acc[:64] * (1/acc[64])      [64, 2048]
  partial = attnT.T @ Wout_shard        [2048, 1024]

The 4-way partial sum over head groups plus b_out is done on the host
(the "all-reduce after out_proj"), as is the batch unshard.

All matmuls use float32r (full fp32 storage, fast PE path).
"""

import os
import numpy as np

B, T, D = 2, 2048, 1024
H, DH = 16, 64
NCORES, GROUPS = 8, 4
HPC = H // GROUPS        # 4 heads per core
F = HPC * DH             # 256 features per core
FT = F // 128            # 2 feature tiles
KTN = D // 128           # 8 contraction tiles
TT = T // 128            # 16 token tiles
NCH = 512                # matmul free-dim chunk
IH = 1024                # attention i-half (psum budget)
VW = DH + 1              # 65: V plus ones column
VF = HPC * VW            # 260: interleaved [V_h | 1] x 4 heads

_prog = None
LAST_RESULT = None


def _build():
    from contextlib import ExitStack

    import concourse.mybir as mybir
    import concourse.tile as tile
    from concourse import bacc
    from concourse.bass import ts

    f32 = mybir.dt.float32
    f32r = mybir.dt.float32r
    f16 = mybir.dt.float16
    Exp = mybir.ActivationFunctionType.Exp

    nc = bacc.Bacc()
    xT = nc.dram_tensor("xT", [D, T], f32r, kind="ExternalInput")
    wq = nc.dram_tensor("wq", [D, F], f32r, kind="ExternalInput")
    wk = nc.dram_tensor("wk", [D, F], f32r, kind="ExternalInput")
    # wv/bv come pre-interleaved from the host: column h*65+64 is a zero
    # weight column whose bias is 1.0, producing the [V_h | 1] layout that
    # supplies the softmax-denominator row of the PV matmul for free.
    wv = nc.dram_tensor("wv", [D, VF], f32r, kind="ExternalInput")
    bq = nc.dram_tensor("bq", [F, 1], f32, kind="ExternalInput")
    bk = nc.dram_tensor("bk", [F, 1], f32, kind="ExternalInput")
    bv = nc.dram_tensor("bv", [1, VF], f32r, kind="ExternalInput")
    wo = nc.dram_tensor("wo", [F, D], f32r, kind="ExternalInput")
    out = nc.dram_tensor("out", [T, D], f32, kind="ExternalOutput")

    with ExitStack() as ctx:
        tc = ctx.enter_context(tile.TileContext(nc))
        pers = ctx.enter_context(tc.tile_pool(name="pers", bufs=1))
        ptp = ctx.enter_context(tc.tile_pool(name="ptp", bufs=2))
        osb = ctx.enter_context(tc.tile_pool(name="osb", bufs=2))
        msc = ctx.enter_context(tc.tile_pool(name="msc", bufs=2))
        psq = ctx.enter_context(tc.tile_pool(name="psq", bufs=2, space="PSUM"))
        pss = ctx.enter_context(tc.tile_pool(name="pss", bufs=2, space="PSUM"))
        pso = ctx.enter_context(tc.tile_pool(name="pso", bufs=1, space="PSUM"))

        xt = pers.tile([128, KTN, T], f32r, tag="xt")
        wqs = pers.tile([128, KTN, F], f32r, tag="wqs")
        wks = pers.tile([128, KTN, F], f32r, tag="wks")
        wvs = pers.tile([128, KTN, VF], f32r, tag="wvs")
        bqc = pers.tile([128, FT, 1], f32, tag="bqc")
        bkc = pers.tile([128, FT, 1], f32, tag="bkc")
        bvr = pers.tile([1, VF], f32r, tag="bvr")
        ones_f = pers.tile([1, 128], f32, tag="ones_f")
        ones = pers.tile([1, 128], f32r, tag="ones")
        wos = pers.tile([128, FT, D], f32r, tag="wos")
        # attention operands in fp16: 1 cyc/row matmuls + FWL weight loads
        qt = pers.tile([128, FT, T], f16, tag="qt")
        kt = pers.tile([128, FT, T], f16, tag="kt")
        vs = pers.tile([128, TT, VF], f16, tag="vs")
        at = pers.tile([128, FT, T], f32r, tag="at")

        # ISA memset can't target f32r; memset f32 then copy-convert
        nc.vector.memset(ones_f[:], 1.0)
        nc.vector.tensor_copy(ones[:], ones_f[:])

        nc.sync.dma_start(bvr[:], bv[:])
        for ft in range(FT):
            nc.sync.dma_start(bqc[:, ft, :], bq[ts(ft, 128), :])
            nc.sync.dma_start(bkc[:, ft, :], bk[ts(ft, 128), :])
        for k in range(KTN):
            nc.sync.dma_start(wqs[:, k, :], wq[ts(k, 128), :])
            nc.sync.dma_start(wks[:, k, :], wk[ts(k, 128), :])
            nc.sync.dma_start(wvs[:, k, :], wv[ts(k, 128), :])
            nc.sync.dma_start(xt[:, k, :], xT[ts(k, 128), :])
        for ft in range(FT):
            nc.sync.dma_start(wos[:, ft, :], wo[ts(ft, 128), :])

        # ---- Q.T / K.T projections: [256 f, 2048 t], f on partitions ----
        for wsb, bcol, dst in ((wqs, bqc, qt), (wks, bkc, kt)):
            for ft in range(FT):
                for c in range(T // NCH):
                    ps = psq.tile([128, NCH], f32, tag="psq")
                    for k in range(KTN):
                        nc.tensor.matmul(
                            ps[:],
                            wsb[:, k, ts(ft, 128)],
                            xt[:, k, ts(c, NCH)],
                            start=(k == 0),
                            stop=(k == KTN - 1),
                        )
                    nc.vector.tensor_scalar_add(
                        dst[:, ft, ts(c, NCH)], ps[:], bcol[:, ft, :]
                    )

        # ---- V projection: [2048 t, 260], tokens on partitions ----
        for t in range(TT):
            pv = psq.tile([128, VF], f32, tag="psq")
            for k in range(KTN):
                nc.tensor.matmul(
                    pv[:], xt[:, k, ts(t, 128)], wvs[:, k, :],
                    start=(k == 0), stop=False,
                )
            # bias via ones-row: V += ones.T @ bv (also writes the 1.0 cols)
            nc.tensor.matmul(pv[:], ones[:, 0:128], bvr[:], start=False, stop=True)
            nc.vector.tensor_copy(vs[:, t, :], pv[:])

        # ---- attention: head pairs (fills both PE row-group halves) ----
        for ic in range(T // NCH):          # 4 i-chunks of 512
            for p in range(FT):             # pair p = heads (2p, 2p+1)
                acc0 = pso.tile([VW, NCH], f32, tag="acc0")
                acc1 = pso.tile([VW, NCH], f32, tag="acc1")
                accs = (acc0, acc1)

                def scores(j):
                    # the two K=64 matmuls use disjoint PE row groups
                    # (partitions 0-63 vs 64-127) -> run concurrently
                    sc = pss.tile([128, 2 * NCH], f32, tag="sc")
                    for hh in range(2):
                        nc.tensor.matmul(
                            sc[:, ts(hh, NCH)],
                            kt[hh * 64: hh * 64 + DH, p, ts(j, 128)],
                            qt[hh * 64: hh * 64 + DH, p, ts(ic, NCH)],
                            start=True, stop=True,
                        )
                    return sc

                sc_cur = scores(0)
                for j in range(TT):
                    pe = ptp.tile([128, 2 * NCH], f16, tag="pe")
                    nc.scalar.activation(pe[:], sc_cur[:], Exp, scale=0.125)
                    if j + 1 < TT:
                        sc_cur = scores(j + 1)  # keep PE busy during exp
                    for hh in range(2):
                        nc.tensor.matmul(
                            accs[hh][:, :],
                            vs[:, j, (2 * p + hh) * VW: (2 * p + hh + 1) * VW],
                            pe[:, ts(hh, NCH)],
                            start=(j == 0), stop=(j == TT - 1),
                        )

                # normalize: attnT = acc[:64] * (1 / acc[64]) , bcast via PE
                for hh in range(2):
                    acc = accs[hh]
                    # custom-DVE ops drop the partition base offset; stage the
                    # denominator row to SBUF partition 0 first
                    dn = msc.tile([1, NCH], f32, tag="dn", bufs=2)
                    nc.vector.tensor_copy(dn[:], acc[DH: DH + 1, :])
                    rc = msc.tile([1, NCH], f32, tag="rc", bufs=2)
                    nc.vector.reciprocal_approx_fast(rc[:], dn[:])
                    rcr = msc.tile([1, NCH], f32r, tag="rcr", bufs=2)
                    nc.vector.tensor_copy(rcr[:], rc[:])  # round to f32r
                    pb = psq.tile([64, NCH], f32, tag="psq")
                    nc.tensor.matmul(
                        pb[:], ones[:, 0:64], rcr[:],
                        start=True, stop=True,
                    )
                    bsb = msc.tile([64, NCH], f32, tag="bsb")
                    nc.vector.tensor_copy(bsb[:], pb[:])
                    dst_sl = ts(ic, NCH)
                    if hh == 0:
                        nc.vector.tensor_mul(
                            at[0:DH, p, dst_sl], acc[0:DH, :], bsb[:]
                        )
                    else:
                        # DVE lanes can't shift partitions; bounce via DMA
                        tmp = msc.tile([DH, NCH], f32r, tag="tmp", bufs=2)
                        nc.vector.tensor_mul(tmp[:], acc[0:DH, :], bsb[:])
                        nc.sync.dma_start(at[64:128, p, dst_sl], tmp[:])

            # ---- out projection for this i-chunk's 4 token tiles ----
            # (interleaved so full-array f32r matmuls pepper the attention
            # stream and the output DMA is spread out)
            for t in range(ic * 4, ic * 4 + 4):
                ob = osb.tile([128, D], f32, tag="ob")
                for c in range(D // NCH):
                    pp = psq.tile([128, NCH], f32, tag="psq")
                    for ft in range(FT):
                        nc.tensor.matmul(
                            pp[:],
                            at[:, ft, ts(t, 128)],
                            wos[:, ft, ts(c, NCH)],
                            start=(ft == 0), stop=(ft == FT - 1),
                        )
                    nc.vector.tensor_copy(ob[:, ts(c, NCH)], pp[:])
                nc.sync.dma_start(out[ts(t, 128), :], ob[:])

    nc.finalize()  # Bacc.compile(): wait legalization, reg alloc, act tables
    return nc


def _get_program():
    global _prog
    if _prog is None:
        _prog = _build()
    return _prog


def kernel(x, W_qkv, b_qkv, W_out, b_out):
    global LAST_RESULT
    from concourse.bass_utils import run_bass_kernel_spmd

    x = np.asarray(x, np.float32)
    W_qkv = np.asarray(W_qkv, np.float32)
    b_qkv = np.asarray(b_qkv, np.float32)
    W_out = np.asarray(W_out, np.float32)
    b_out = np.asarray(b_out, np.float32)

    nc = _get_program()

    in_maps = []
    for c in range(NCORES):
        b, g = divmod(c, GROUPS)
        sl = slice(g * F, (g + 1) * F)
        # interleave Wv/bv with [zero-weight, bias=1] columns at h*65+64
        wv_g = W_qkv[:, 2 * D:3 * D][:, sl]
        bv_g = b_qkv[2 * D:3 * D][sl]
        wv_i = np.zeros((D, VF), np.float32)
        bv_i = np.zeros((1, VF), np.float32)
        for h in range(HPC):
            wv_i[:, h * VW: h * VW + DH] = wv_g[:, h * DH:(h + 1) * DH]
            bv_i[0, h * VW: h * VW + DH] = bv_g[h * DH:(h + 1) * DH]
            bv_i[0, h * VW + DH] = 1.0
        in_maps.append({
            "xT": np.ascontiguousarray(x[b].T),
            "wq": np.ascontiguousarray(W_qkv[:, 0 * D:1 * D][:, sl]),
            "wk": np.ascontiguousarray(W_qkv[:, 1 * D:2 * D][:, sl]),
            "wv": wv_i,
            "bq": np.ascontiguousarray(b_qkv[0 * D:1 * D][sl][:, None]),
            "bk": np.ascontiguousarray(b_qkv[1 * D:2 * D][sl][:, None]),
            "bv": bv_i,
            "wo": np.ascontiguousarray(W_out[sl, :]),
        })

    kw = {}
    if os.environ.get("KERNEL_TRACE") == "1":
        kw["trace"] = True
    res = run_bass_kernel_spmd(nc, in_maps, core_ids=list(range(NCORES)), **kw)
    LAST_RESULT = res

    out = np.empty((B, T, D), np.float32)
    for b in range(B):
        acc = res.results[GROUPS * b]["out"].astype(np.float32)
        for g in range(1, GROUPS):
            acc = acc + res.results[GROUPS * b + g]["out"]
        out[b] = acc + b_out
    return out
